# revision 12
# baseline (speedup 1.0000x reference)
"""Trainium2 Bass kernel for nn_MultiHeadGATLayerMerged (v2).

Math (reference semantics):
  Wh[d,h] = x @ W[d,h]                                   (per batch b)
  e_src[d] = x @ (W[d,H-1] @ a[d,H-1,:OUT])              (only last head's
  e_dst[d] = x @ (W[d,H-1] @ a[d,H-1,OUT:])               logits survive)
  z_d[i,j] = e_src[d][i] + e_dst[d][j]
  e[i,j]   = z_{d*}[i,j],  d* = max d with A_d[i,j] != 0, else -inf
  P = exp(leakyrelu(e)); alpha = P / rowsum(P)
  out = (1/H) * diag(1/rowsum) * sum_d (A_d o P) @ (x @ (W[d,0]+W[d,1]))

Kernel strategy (one NeuronCore per batch element, 8 cores):
  Attention plane held TRANSPOSED (j on partitions, i free) so masked planes
  feed the PE as operands without transposes.

  Priority merge via uint16 keys (replaces slow 1x copy_predicated):
    u_d = round((z_d + 16)*64 + 16384*d) * m_d      -- one fused
          scalar_tensor_tensor (2x mode) per direction
    umax = max-tree(u_0..u_3)                        -- priority = high bits
    ukey = umax & 0x3fff                             -- strip direction bits
    P    = Exp(Lrelu(ukey/64 - 16))                  -- decode inside ACT
  Off-union elements decode to exp(lrelu(-16)) = exp(-0.16) instead of 0;
  a host-precomputed per-row count corrects the rowsum, and S_d = (u_d>0)*P
  zeroes them in the matmuls.

  Main matmuls accumulate out^T (o on partitions): whs stationary (2 LDW per
  (jc,d) instead of 8), S streamed at n=512. Host transposes the result.
"""

import numpy as np
import ml_dtypes
from contextlib import ExitStack

import concourse.bass as bass
import concourse.mybir as mybir
import concourse.tile as tile
from concourse.bass_utils import run_bass_kernel_spmd

dt = mybir.dt
AF = mybir.ActivationFunctionType
ALU = mybir.AluOpType

B, N, F, OUT, H, D = 8, 1024, 256, 256, 2, 4
P = 128
NJC = N // P   # j chunks (partition chunks of transposed plane)
FC = F // P    # f chunks for contraction

USCALE = 64.0       # logit quantization scale
UBIAS = 16.0        # logit shift (z + UBIAS > 0 guaranteed)
UBASE = 2048.0      # keeps encoded keys in fp16-normal bit range
DSTEP = 8192.0      # per-direction priority offset (bits 13..14)
DECODE_BIAS = -(UBIAS + UBASE / USCALE)          # -48
P_ABSENT = float(np.float16(np.exp(np.float32(0.01 * DECODE_BIAS))))  # u=0 decode


# ---------------------------------------------------------------------------
# BIR post-pass: some walrus instruction structs have few semaphore-wait
# slots. Hoist waits into standalone EventSemaphore instructions placed
# immediately before the owning instruction (same engine, program order =>
# semantics identical: waits are monotone thresholds).
_ZERO_WAIT_TYPES = (mybir.InstCopyPredicated,)
_DEFAULT_LIMIT = 1


def _hoist_waits(nc):
    n_fixed = 0
    for fn in nc.m.functions:
        for bb in fn.blocks:
            insts = list(bb.instructions)
            new_insts = []
            for inst in insts:
                si = getattr(inst, "sync_info", None)
                if si is not None and si.on_wait:
                    limit = 0 if isinstance(inst, _ZERO_WAIT_TYPES) else _DEFAULT_LIMIT
                    waits = list(si.on_wait)
                    if len(waits) > limit:
                        excess = waits[: len(waits) - limit] if limit else waits
                        keep = waits[len(waits) - limit:] if limit else []
                        for k in range(0, len(excess), 2):
                            ev = mybir.InstEventSemaphore(
                                name=f"{inst.name}-hw{k}", ins=[], outs=[])
                            ev.engine = inst.engine
                            ev.debug = inst.debug
                            ev.sync_info = mybir.SyncInfo(
                                on_wait=excess[k:k + 2], on_update=[])
                            new_insts.append(ev)
                        inst.sync_info = mybir.SyncInfo(
                            on_wait=keep, on_update=list(si.on_update))
                        n_fixed += 1
                new_insts.append(inst)
            bb.instructions = new_insts
    return n_fixed


def _dedupe_ldweights(nc):
    """Drop LDWEIGHTS identical to the immediately preceding LDWEIGHTS on PE
    (stationary operand persists across matmuls)."""
    n_drop = 0
    for fn in nc.m.functions:
        for bb in fn.blocks:
            insts = list(bb.instructions)
            new_insts = []
            prev_key = None
            for inst in insts:
                if isinstance(inst, mybir.InstLdweights):
                    key = repr(inst.ins[0])
                    si = getattr(inst, "sync_info", None)
                    clean = si is None or (not si.on_wait and not si.on_update)
                    if key == prev_key and clean:
                        n_drop += 1
                        continue
                    prev_key = key
                elif isinstance(inst, (mybir.InstMatmult, mybir.InstEventSemaphore)):
                    pass
                else:
                    if getattr(inst, "engine", None) == mybir.EngineType.PE:
                        prev_key = None
                new_insts.append(inst)
            bb.instructions = new_insts
    return n_drop


def _build(nc: bass.Bass):
    xT = nc.dram_tensor("xT", [F, N], dt.float32, kind="ExternalInput")
    mU16 = nc.dram_tensor("mU16", [D, N, N], dt.uint16, kind="ExternalInput")
    wvec = nc.dram_tensor("wvec", [F, 2 * D], dt.float32, kind="ExternalInput")
    ws = nc.dram_tensor("ws", [D, F, OUT], dt.float32, kind="ExternalInput")
    outT = nc.dram_tensor("outT", [OUT, N], dt.float32, kind="ExternalOutput")
    rsum = nc.dram_tensor("rsum", [1, N], dt.float32, kind="ExternalOutput")

    with tile.TileContext(nc) as tc, ExitStack() as ctx:
        cpool = ctx.enter_context(tc.tile_pool(name="consts", bufs=1))
        spool = ctx.enter_context(tc.tile_pool(name="statics", bufs=1))

        onesf = cpool.tile([1, P], dt.float32, name="onesf", tag="onesf")
        nc.vector.memset(onesf[:], 1.0)
        onesfr = cpool.tile([1, P], dt.float32r, name="onesfr", tag="onesfr")
        nc.vector.tensor_copy(onesfr[:], onesf[:])
        onescol16 = cpool.tile([P, 1], dt.float16, name="onescol16", tag="onescol16")
        nc.vector.memset(onescol16[:], 1.0)
        biasm = cpool.tile([P, 1], dt.float32, name="biasm", tag="biasm")
        nc.vector.memset(biasm[:], DECODE_BIAS)

        # ---- load x^T (f on partitions), make fp32r copy for projections
        xtr = []
        with tc.tile_pool(name="xstage", bufs=2) as xstage:
            for fc in range(FC):
                t = xstage.tile([P, N], dt.float32, name=f"xt{fc}", tag="xts")
                nc.sync.dma_start(t[:], xT[fc * P:(fc + 1) * P, :])
                tr = spool.tile([P, N], dt.float32r, name=f"xtr{fc}", tag=f"xtr{fc}")
                nc.vector.tensor_copy(tr[:], t[:])
                xtr.append(tr)

        # ---- load wvec chunks (src columns pre-scaled by USCALE on host)
        wv = []
        for fc in range(FC):
            t = spool.tile([P, 2 * D], dt.float32, name=f"wv{fc}", tag=f"wv{fc}")
            nc.sync.dma_start(t[:], wvec[fc * P:(fc + 1) * P, :])
            wv.append(t)

        # ---- load Wsum, round to fp32r
        wsr = [[None] * FC for _ in range(D)]
        with tc.tile_pool(name="wstage", bufs=2) as wstage:
            for d in range(D):
                for fc in range(FC):
                    t = wstage.tile([P, OUT], dt.float32, name=f"ws{d}{fc}", tag="wss")
                    nc.sync.dma_start(t[:], ws[d, fc * P:(fc + 1) * P, :])
                    tr = spool.tile([P, OUT], dt.float32r, name=f"wsr{d}{fc}", tag=f"wsr{d}{fc}")
                    nc.vector.tensor_copy(tr[:], t[:])
                    wsr[d][fc] = tr

        # ---- src rows 64*src (1, N) f32 per direction (M=1 matmuls)
        wvr = []
        for fc in range(FC):
            t = spool.tile([P, 2 * D], dt.float32r, name=f"wvr{fc}", tag=f"wvr{fc}")
            nc.vector.tensor_copy(t[:], wv[fc][:])
            wvr.append(t)
        srowpool = ExitStack()
        srowp = srowpool.enter_context(tc.tile_pool(name="srowp", bufs=1))
        src_row = []
        with tc.tile_pool(name="srcps", bufs=2, space="PSUM") as srcps:
            for d in range(D):
                ps = srcps.tile([1, N], dt.float32, name=f"sps{d}", tag="sps")
                for hhalf in range(2):
                    sl = slice(hhalf * 512, (hhalf + 1) * 512)
                    for fc in range(FC):
                        nc.tensor.matmul(
                            ps[:, sl], wvr[fc][:, d:d + 1], xtr[fc][:, sl],
                            start=(fc == 0), stop=(fc == FC - 1))
                t = srowp.tile([1, N], dt.float32, name=f"srcrow{d}", tag=f"srcrow{d}")
                nc.scalar.copy(t[:], ps[:])
                src_row.append(t)

        # ---- dst columns -> u offsets: 64*dst + 64*UBIAS + DSTEP*d  (f32)
        dstoff = []
        with tc.tile_pool(name="dstps", bufs=2, space="PSUM") as dstps:
            for jc in range(NJC):
                ps = dstps.tile([P, D], dt.float32, name=f"dps{jc}", tag="dps")
                for fc in range(FC):
                    nc.tensor.matmul(
                        ps[:], xtr[fc][:, jc * P:(jc + 1) * P], wvr[fc][:, D:2 * D],
                        start=(fc == 0), stop=(fc == FC - 1))
                t = spool.tile([P, D], dt.float32, name=f"dstoff{jc}", tag=f"dstoff{jc}")
                for d in range(D):
                    nc.scalar.activation(
                        t[:, d:d + 1], ps[:, d:d + 1], AF.Copy,
                        bias=float(USCALE * UBIAS + UBASE + DSTEP * d),
                        scale=float(USCALE))
                dstoff.append(t)

        # ---- srcb64_d: (128, N) broadcast of 64*src_row[d] down partitions
        srcb = []
        with tc.tile_pool(name="bcps", bufs=2, space="PSUM") as bcps:
            for d in range(D):
                sr = srowp.tile([1, N], dt.float32r, name=f"srcr{d}", tag=f"srcr{d}")
                nc.vector.tensor_copy(sr[:], src_row[d][:])
                ps = bcps.tile([P, N], dt.float32, name=f"bps{d}", tag="bps")
                for hhalf in range(2):
                    sl = slice(hhalf * 512, (hhalf + 1) * 512)
                    nc.tensor.matmul(ps[:, sl], onesfr[:], sr[:, sl],
                                     start=True, stop=True)
                t = spool.tile([P, N], dt.float16, name=f"srcb{d}", tag=f"srcb{d}")
                nc.scalar.copy(t[:], ps[:])
                srcb.append(t)
        srowpool.close()

        # ---- persistent PSUM: out^T accumulators (o on partitions) + rowsum
        outps_pool = ctx.enter_context(tc.tile_pool(name="outps", bufs=1, space="PSUM"))
        outT_ps = [outps_pool.tile([P, N], dt.float32, name=f"otps{ob}", tag=f"otps{ob}")
                   for ob in range(2)]
        rs_ps = outps_pool.tile([1, N], dt.float32, name="rsps", tag="rsps")

        # ---- streaming pools for the main loop
        loopctx = ctx.enter_context(ExitStack())
        mpool = loopctx.enter_context(tc.tile_pool(name="masks", bufs=5))
        upool = loopctx.enter_context(tc.tile_pool(name="us", bufs=2))
        umpool = loopctx.enter_context(tc.tile_pool(name="ums", bufs=2))
        tpool = loopctx.enter_context(tc.tile_pool(name="tmax", bufs=2))
        epool = loopctx.enter_context(tc.tile_pool(name="es", bufs=8))
        elpool = loopctx.enter_context(tc.tile_pool(name="elrs", bufs=4))
        ptpool = loopctx.enter_context(tc.tile_pool(name="ps16", bufs=8))
        spool2 = loopctx.enter_context(tc.tile_pool(name="ss", bufs=3))
        wpool = loopctx.enter_context(tc.tile_pool(name="whsp", bufs=8))
        projps = loopctx.enter_context(tc.tile_pool(name="projps", bufs=2, space="PSUM"))

        GRP = 4
        ngrp = NJC // GRP
        mfs = {}     # (jc, d) -> mask tile (uint16 0/1)
        us = {}      # (jc, d) -> u tile
        umax = {}    # jc -> merged key tile (uint16, masked to 0x3fff)
        pt = {}      # jc -> P tile fp16
        whs = {}     # (jc, d) -> whs tile fp16

        def stage_a(grp):
            """DMA masks, projections, u-build + max-merge for group grp."""
            jcs = tuple(range(GRP * grp, GRP * grp + GRP))
            for jc in jcs:
                jsl = slice(jc * P, (jc + 1) * P)
                for d in range(D):
                    t16 = mpool.tile([P, N], dt.uint16, name=f"mf{d}", tag=f"mf{d}")
                    nc.sync.dma_start(t16[:], mU16[d, jsl, :])
                    mfs[(jc, d)] = t16

            # projections whs[jc,d] (keeps PE warm between attention bursts)
            for jc in jcs:
                for d0 in (0, 2):
                    pss = [projps.tile([P, OUT], dt.float32, name=f"pps{d0+k}",
                                       tag="pps") for k in range(2)]
                    for fc in range(FC):
                        for k in range(2):
                            nc.tensor.matmul(
                                pss[k][:], xtr[fc][:, jc * P:(jc + 1) * P],
                                wsr[d0 + k][fc][:],
                                start=(fc == 0), stop=(fc == FC - 1),
                                skip_group_check=True)
                    for k in range(2):
                        t = wpool.tile([P, OUT], dt.float16, name=f"whs{d0+k}",
                                       tag=f"whs{d0+k}")
                        nc.scalar.copy(t[:], pss[k][:])
                        whs[(jc, d0 + k)] = t

            # u_d = ((srcb64_d + dstoff_d) * m_d) -> uint16   [DVE, fused]
            for jc in jcs:
                for d in range(D):
                    u = upool.tile([P, N], dt.uint16, name=f"u{d}", tag=f"u{d}")
                    nc.vector.scalar_tensor_tensor(
                        u[:], srcb[d][:], dstoff[jc][:, d:d + 1], mfs[(jc, d)][:],
                        op0=ALU.add, op1=ALU.mult)
                    us[(jc, d)] = u

            # max-tree as fp16 (bit order == value order for these keys);
            # 32-bit AND strips the direction bits (bitwise is 32-bit-only
            # on DVE, and Pool supports no TensorTensor at all).
            for jc in jcs:
                t01 = tpool.tile([P, N], dt.float16, name="t01", tag="t01")
                nc.vector.tensor_tensor(t01[:], us[(jc, 0)][:].bitcast(dt.float16),
                                        us[(jc, 1)][:].bitcast(dt.float16), ALU.max)
                t23 = tpool.tile([P, N], dt.float16, name="t23", tag="t23")
                nc.vector.tensor_tensor(t23[:], us[(jc, 2)][:].bitcast(dt.float16),
                                        us[(jc, 3)][:].bitcast(dt.float16), ALU.max)
                um = umpool.tile([P, N], dt.float16, name="um", tag="um")
                nc.vector.tensor_tensor(um[:], t01[:], t23[:], ALU.max)
                uk = epool.tile([P, N], dt.uint16, name="uk", tag="uk")
                nc.vector.tensor_scalar(uk[:].bitcast(dt.uint32),
                                        um[:].bitcast(dt.uint32), 0x1FFF1FFF, None,
                                        op0=ALU.bitwise_and)
                umax[jc] = uk

        def stage_act(grp):
            """Lrelu-decode + Exp for group grp (batched per function)."""
            jcs = tuple(range(GRP * grp, GRP * grp + GRP))
            elr = {}
            for jc in jcs:
                t = elpool.tile([P, N], dt.float16, name="elr", tag="elr")
                nc.scalar.activation(t[:], umax[jc][:], AF.Lrelu,
                                     bias=biasm[:], scale=1.0 / USCALE, alpha=0.01)
                elr[jc] = t
            for jc in jcs:
                p16 = ptpool.tile([P, N], dt.float16, name="pt", tag="pt")
                nc.scalar.activation(p16[:], elr[jc][:], AF.Exp, bias=0.0,
                                     scale=1.0)
                pt[jc] = p16

        def stage_b(grp):
            """rowsum + S-mults + main matmuls for group grp."""
            jcs = tuple(range(GRP * grp, GRP * grp + GRP))
            for jc in jcs:
                # rowsum row: rs[0, i] += sum_j P^T[j, i]
                for ih in range(2):
                    isl = slice(ih * 512, (ih + 1) * 512)
                    nc.tensor.matmul(rs_ps[:, isl], onescol16[:], pt[jc][:, isl],
                                     start=(jc == 0),
                                     stop=(jc == NJC - 1),
                                     skip_group_check=True)

                # S_d = (u_d > 0) * P  [DVE stt], then out^T matmuls
                for d in range(D):
                    s = spool2.tile([P, N], dt.float16, name=f"s{d}", tag="s")
                    nc.vector.tensor_tensor(
                        s[:], mfs[(jc, d)][:], pt[jc][:], ALU.mult)
                    for ob in range(2):
                        osl = slice(ob * P, (ob + 1) * P)
                        for ih in range(2):
                            isl = slice(ih * 512, (ih + 1) * 512)
                            nc.tensor.matmul(
                                outT_ps[ob][:, isl],
                                whs[(jc, d)][:, osl], s[:, isl],
                                start=(jc == 0 and d == 0),
                                stop=(jc == NJC - 1 and d == D - 1),
                                skip_group_check=True)

        stage_a(0)
        for grp in range(ngrp):
            if grp + 1 < ngrp:
                stage_a(grp + 1)
            stage_act(grp)
            stage_b(grp)

        # ---- epilogue: raw out^T and raw rowsum to HBM; host does the
        # 0.5/(rowsum - corr) scaling and the transpose (free).
        loopctx.close()
        with tc.tile_pool(name="epil", bufs=1) as epil:
            rs_row = epil.tile([1, N], dt.float32, name="rsrow", tag="rsrow")
            nc.vector.tensor_copy(rs_row[:], rs_ps[:])
            nc.sync.dma_start(rsum[:, :], rs_row[:])
            for ob in range(2):
                o = epil.tile([P, N], dt.float32, name=f"osb{ob}", tag=f"osb{ob}")
                nc.scalar.copy(o[:], outT_ps[ob][:])
                nc.sync.dma_start(outT[ob * P:(ob + 1) * P, :], o[:])

    return nc


_CACHED = {}


def _get_nc():
    if "nc" not in _CACHED:
        nc = bass.Bass()
        _build(nc)
        _hoist_waits(nc)
        _dedupe_ldweights(nc)
        _CACHED["nc"] = nc
    return _CACHED["nc"]


def _prep_host(x, A_U, A_D, A_R, A_L, W, a):
    x = np.asarray(x, dtype=np.float32)
    W = np.asarray(W, dtype=np.float32)
    a = np.asarray(a, dtype=np.float32)

    masks = [np.asarray(m) for m in (A_U, A_D, A_R, A_L)]
    # transposed masks (j on rows): mT[d][j, i] = A_d[i, j]
    mT = np.stack([np.ascontiguousarray(m.T) for m in masks])
    m_u16 = (mT != 0).astype(np.uint16)

    # rowsum correction: off-union elements decode to P_ABSENT instead of 0
    union = (m_u16[0] | m_u16[1] | m_u16[2] | m_u16[3]).astype(bool)
    absent_cnt = (~union).sum(axis=0).astype(np.float64)      # per column i
    rcorr = (P_ABSENT * absent_cnt)                           # (N,)

    # attention vector folding (last head only survives the merge);
    # src columns pre-scaled by USCALE for the u encoding
    wv_cols = [USCALE * (W[d, H - 1] @ a[d, H - 1, :OUT]) for d in range(D)] + \
              [W[d, H - 1] @ a[d, H - 1, OUT:] for d in range(D)]
    wvec = np.stack(wv_cols, axis=1).astype(np.float32)   # (F, 2D)
    ws = np.ascontiguousarray(W.sum(axis=1), dtype=np.float32)  # (D, F, OUT)
    return x, m_u16, wvec, ws, rcorr


def kernel(x, A_U, A_D, A_R, A_L, W, a):
    x, m_u16, wvec, ws, rcorr = _prep_host(x, A_U, A_D, A_R, A_L, W, a)

    nc = _get_nc()
    core_ids = list(range(B))
    in_maps = []
    for b in range(B):
        in_maps.append({
            "xT": np.ascontiguousarray(x[b].T),
            "mU16": m_u16,
            "wvec": wvec,
            "ws": ws,
        })
    res = run_bass_kernel_spmd(nc, in_maps, core_ids)
    outs = []
    for b in range(B):
        ot = res.results[b]["outT"].astype(np.float64)     # (OUT, N) raw
        rs = res.results[b]["rsum"].astype(np.float64)[0]  # (N,) raw
        inv = 0.5 / (rs - rcorr)
        outs.append((ot * inv[None, :]).T)
    return np.stack(outs, axis=0).astype(np.float32)


# revision 14
# speedup vs baseline: 1.2851x; 1.2851x over previous
"""Trainium2 Bass kernel for nn_MultiHeadGATLayerMerged (v2).

Math (reference semantics):
  Wh[d,h] = x @ W[d,h]                                   (per batch b)
  e_src[d] = x @ (W[d,H-1] @ a[d,H-1,:OUT])              (only last head's
  e_dst[d] = x @ (W[d,H-1] @ a[d,H-1,OUT:])               logits survive)
  z_d[i,j] = e_src[d][i] + e_dst[d][j]
  e[i,j]   = z_{d*}[i,j],  d* = max d with A_d[i,j] != 0, else -inf
  P = exp(leakyrelu(e)); alpha = P / rowsum(P)
  out = (1/H) * diag(1/rowsum) * sum_d (A_d o P) @ (x @ (W[d,0]+W[d,1]))

Kernel strategy (one NeuronCore per batch element, 8 cores):
  Attention plane held TRANSPOSED (j on partitions, i free) so masked planes
  feed the PE as operands without transposes.

  Work split per 128-row tile of the transposed plane:
    ACT: z_d = Identity(srcb_d, bias=dstcol_d)  (4 passes; Copy/Identity need
         no LUT so only Lrelu/Exp swap tables), Lrelu, Exp
    DVE: priority merge (copy_predicated x4 into a -3000 default), S_d = m*P
    Pool: e-tile memsets
    PE:  projections (fp32r), rowsum via ones-column matmuls, main matmuls
  Main matmuls accumulate out^T (o on partitions): whs stationary (2 LDW per
  (jc,d) instead of 8), S streamed at n=512. The final 0.5/rowsum scaling and
  the transpose happen on the host (device ships raw out^T and rowsum).
"""

import numpy as np
import ml_dtypes
from contextlib import ExitStack

import concourse.bass as bass
import concourse.mybir as mybir
import concourse.tile as tile
from concourse.bass_utils import run_bass_kernel_spmd

dt = mybir.dt
AF = mybir.ActivationFunctionType
ALU = mybir.AluOpType

B, N, F, OUT, H, D = 8, 1024, 256, 256, 2, 4
P = 128
NJC = N // P   # j chunks (partition chunks of transposed plane)
FC = F // P    # f chunks for contraction

E_DEFAULT = -3000.0   # merge default; lrelu -> -30, exp -> 0 in fp16


# ---------------------------------------------------------------------------
# BIR post-pass: some walrus instruction structs have few semaphore-wait
# slots. Hoist waits into standalone EventSemaphore instructions placed
# immediately before the owning instruction (same engine, program order =>
# semantics identical: waits are monotone thresholds).
_ZERO_WAIT_TYPES = (mybir.InstCopyPredicated,)
_DEFAULT_LIMIT = 1


def _hoist_waits(nc):
    n_fixed = 0
    for fn in nc.m.functions:
        for bb in fn.blocks:
            insts = list(bb.instructions)
            new_insts = []
            for inst in insts:
                si = getattr(inst, "sync_info", None)
                if si is not None and si.on_wait:
                    limit = 0 if isinstance(inst, _ZERO_WAIT_TYPES) else _DEFAULT_LIMIT
                    waits = list(si.on_wait)
                    if len(waits) > limit:
                        excess = waits[: len(waits) - limit] if limit else waits
                        keep = waits[len(waits) - limit:] if limit else []
                        for k in range(0, len(excess), 2):
                            ev = mybir.InstEventSemaphore(
                                name=f"{inst.name}-hw{k}", ins=[], outs=[])
                            ev.engine = inst.engine
                            ev.debug = inst.debug
                            ev.sync_info = mybir.SyncInfo(
                                on_wait=excess[k:k + 2], on_update=[])
                            new_insts.append(ev)
                        inst.sync_info = mybir.SyncInfo(
                            on_wait=keep, on_update=list(si.on_update))
                        n_fixed += 1
                new_insts.append(inst)
            bb.instructions = new_insts
    return n_fixed


def _dedupe_ldweights(nc):
    """Drop LDWEIGHTS identical to the immediately preceding LDWEIGHTS on PE
    (stationary operand persists across matmuls)."""
    n_drop = 0
    for fn in nc.m.functions:
        for bb in fn.blocks:
            insts = list(bb.instructions)
            new_insts = []
            prev_key = None
            for inst in insts:
                if isinstance(inst, mybir.InstLdweights):
                    key = repr(inst.ins[0])
                    si = getattr(inst, "sync_info", None)
                    clean = si is None or (not si.on_wait and not si.on_update)
                    if key == prev_key and clean:
                        n_drop += 1
                        continue
                    prev_key = key
                elif isinstance(inst, (mybir.InstMatmult, mybir.InstEventSemaphore)):
                    pass
                else:
                    if getattr(inst, "engine", None) == mybir.EngineType.PE:
                        prev_key = None
                new_insts.append(inst)
            bb.instructions = new_insts
    return n_drop


def _build(nc: bass.Bass):
    xT = nc.dram_tensor("xT", [F, N], dt.float32, kind="ExternalInput")
    mF16 = nc.dram_tensor("mF16", [D, N, N], dt.float16, kind="ExternalInput")
    wvec = nc.dram_tensor("wvec", [F, 2 * D], dt.float32, kind="ExternalInput")
    ws = nc.dram_tensor("ws", [D, F, OUT], dt.float32, kind="ExternalInput")
    outT = nc.dram_tensor("outT", [OUT, N], dt.float32, kind="ExternalOutput")
    rsum = nc.dram_tensor("rsum", [1, N], dt.float32, kind="ExternalOutput")

    with tile.TileContext(nc) as tc, ExitStack() as ctx:
        cpool = ctx.enter_context(tc.tile_pool(name="consts", bufs=1))
        spool = ctx.enter_context(tc.tile_pool(name="statics", bufs=1))

        onesf = cpool.tile([1, P], dt.float32, name="onesf", tag="onesf")
        nc.vector.memset(onesf[:], 1.0)
        onesfr = cpool.tile([1, P], dt.float32r, name="onesfr", tag="onesfr")
        nc.vector.tensor_copy(onesfr[:], onesf[:])
        onescol16 = cpool.tile([P, 1], dt.float16, name="onescol16", tag="onescol16")
        nc.vector.memset(onescol16[:], 1.0)

        # ---- load x^T (f on partitions), make fp32r copy for projections
        xtr = []
        with tc.tile_pool(name="xstage", bufs=2) as xstage:
            for fc in range(FC):
                t = xstage.tile([P, N], dt.float32, name=f"xt{fc}", tag="xts")
                nc.sync.dma_start(t[:], xT[fc * P:(fc + 1) * P, :])
                tr = spool.tile([P, N], dt.float32r, name=f"xtr{fc}", tag=f"xtr{fc}")
                nc.vector.tensor_copy(tr[:], t[:])
                xtr.append(tr)

        # ---- load wvec chunks
        wv = []
        for fc in range(FC):
            t = spool.tile([P, 2 * D], dt.float32, name=f"wv{fc}", tag=f"wv{fc}")
            nc.sync.dma_start(t[:], wvec[fc * P:(fc + 1) * P, :])
            wv.append(t)

        # ---- load Wsum, round to fp32r
        wsr = [[None] * FC for _ in range(D)]
        with tc.tile_pool(name="wstage", bufs=2) as wstage:
            for d in range(D):
                for fc in range(FC):
                    t = wstage.tile([P, OUT], dt.float32, name=f"ws{d}{fc}", tag="wss")
                    nc.sync.dma_start(t[:], ws[d, fc * P:(fc + 1) * P, :])
                    tr = spool.tile([P, OUT], dt.float32r, name=f"wsr{d}{fc}", tag=f"wsr{d}{fc}")
                    nc.vector.tensor_copy(tr[:], t[:])
                    wsr[d][fc] = tr

        # ---- src rows 64*src (1, N) f32 per direction (M=1 matmuls)
        wvr = []
        for fc in range(FC):
            t = spool.tile([P, 2 * D], dt.float32r, name=f"wvr{fc}", tag=f"wvr{fc}")
            nc.vector.tensor_copy(t[:], wv[fc][:])
            wvr.append(t)
        srowpool = ExitStack()
        srowp = srowpool.enter_context(tc.tile_pool(name="srowp", bufs=1))
        src_row = []
        with tc.tile_pool(name="srcps", bufs=2, space="PSUM") as srcps:
            for d in range(D):
                ps = srcps.tile([1, N], dt.float32, name=f"sps{d}", tag="sps")
                for hhalf in range(2):
                    sl = slice(hhalf * 512, (hhalf + 1) * 512)
                    for fc in range(FC):
                        nc.tensor.matmul(
                            ps[:, sl], wvr[fc][:, d:d + 1], xtr[fc][:, sl],
                            start=(fc == 0), stop=(fc == FC - 1))
                t = srowp.tile([1, N], dt.float32, name=f"srcrow{d}", tag=f"srcrow{d}")
                nc.scalar.copy(t[:], ps[:])
                src_row.append(t)

        # ---- dst columns (128, D) per jc
        dst_col = []
        with tc.tile_pool(name="dstps", bufs=2, space="PSUM") as dstps:
            for jc in range(NJC):
                ps = dstps.tile([P, D], dt.float32, name=f"dps{jc}", tag="dps")
                for fc in range(FC):
                    nc.tensor.matmul(
                        ps[:], xtr[fc][:, jc * P:(jc + 1) * P], wvr[fc][:, D:2 * D],
                        start=(fc == 0), stop=(fc == FC - 1))
                t = spool.tile([P, D], dt.float32, name=f"dstcol{jc}", tag=f"dstcol{jc}")
                nc.scalar.copy(t[:], ps[:])
                dst_col.append(t)

        # ---- srcb_d: (128, N) broadcast of src_row[d] down partitions (fp16)
        srcb = []
        with tc.tile_pool(name="bcps", bufs=2, space="PSUM") as bcps:
            for d in range(D):
                sr = srowp.tile([1, N], dt.float32r, name=f"srcr{d}", tag=f"srcr{d}")
                nc.vector.tensor_copy(sr[:], src_row[d][:])
                ps = bcps.tile([P, N], dt.float32, name=f"bps{d}", tag="bps")
                for hhalf in range(2):
                    sl = slice(hhalf * 512, (hhalf + 1) * 512)
                    nc.tensor.matmul(ps[:, sl], onesfr[:], sr[:, sl],
                                     start=True, stop=True)
                t = spool.tile([P, N], dt.float16, name=f"srcb{d}", tag=f"srcb{d}")
                nc.scalar.copy(t[:], ps[:])
                srcb.append(t)
        srowpool.close()

        # ---- persistent PSUM: out^T accumulators (o on partitions) + rowsum
        outps_pool = ctx.enter_context(tc.tile_pool(name="outps", bufs=1, space="PSUM"))
        outT_ps = [outps_pool.tile([P, N], dt.float32, name=f"otps{ob}", tag=f"otps{ob}")
                   for ob in range(2)]
        rs_ps = outps_pool.tile([1, N], dt.float32, name="rsps", tag="rsps")

        # ---- streaming pools for the main loop
        loopctx = ctx.enter_context(ExitStack())
        mpool = loopctx.enter_context(tc.tile_pool(name="masks", bufs=6))
        zpool = loopctx.enter_context(tc.tile_pool(name="zs", bufs=2))
        epool = loopctx.enter_context(tc.tile_pool(name="es", bufs=8))
        ptpool = loopctx.enter_context(tc.tile_pool(name="ps16", bufs=8))
        spool2 = loopctx.enter_context(tc.tile_pool(name="ss", bufs=3))
        wpool = loopctx.enter_context(tc.tile_pool(name="whsp", bufs=8))
        projps = loopctx.enter_context(tc.tile_pool(name="projps", bufs=2, space="PSUM"))

        GRP = 4
        ngrp = NJC // GRP
        mfs = {}     # (jc, d) -> mask tile (fp16 0/1)
        zs = {}      # (jc, d) -> z tile fp16
        es = {}      # jc -> merged logits tile fp16
        pt = {}      # jc -> P tile fp16
        whs = {}     # (jc, d) -> whs tile fp16

        def stage_dma(grp):
            jcs = tuple(range(GRP * grp, GRP * grp + GRP))
            for jc in jcs:
                jsl = slice(jc * P, (jc + 1) * P)
                for d in range(D):
                    t16 = mpool.tile([P, N], dt.float16, name=f"mf{d}", tag=f"mf{d}")
                    nc.sync.dma_start(t16[:], mF16[d, jsl, :])
                    mfs[(jc, d)] = t16

        def stage_proj(grp):
            """projections whs[jc,d] on PE + e-default memsets on Pool."""
            jcs = tuple(range(GRP * grp, GRP * grp + GRP))
            for jc in jcs:
                e = epool.tile([P, N], dt.float16, name="e", tag="e")
                nc.gpsimd.memset(e[:], E_DEFAULT)
                es[jc] = e
            for jc in jcs:
                for d0 in (0, 2):
                    pss = [projps.tile([P, OUT], dt.float32, name=f"pps{d0+k}",
                                       tag="pps") for k in range(2)]
                    for fc in range(FC):
                        for k in range(2):
                            nc.tensor.matmul(
                                pss[k][:], xtr[fc][:, jc * P:(jc + 1) * P],
                                wsr[d0 + k][fc][:],
                                start=(fc == 0), stop=(fc == FC - 1),
                                skip_group_check=True)
                    for k in range(2):
                        t = wpool.tile([P, OUT], dt.float16, name=f"whs{d0+k}",
                                       tag=f"whs{d0+k}")
                        nc.scalar.copy(t[:], pss[k][:])
                        whs[(jc, d0 + k)] = t

        def stage_z(grp):
            """z_d = srcb_d + dstcol_d on ACT (Identity with per-partition
            bias; Copy/Identity are tableless so no LUT swaps)."""
            jcs = tuple(range(GRP * grp, GRP * grp + GRP))
            for jc in jcs:
                for d in range(D):
                    z = zpool.tile([P, N], dt.float16, name=f"z{d}", tag=f"z{d}")
                    nc.scalar.activation(z[:], srcb[d][:], AF.Identity,
                                         bias=dst_col[jc][:, d:d + 1], scale=1.0)
                    zs[(jc, d)] = z

        def stage_merge(grp):
            """priority merge on DVE: later d overwrites where mask != 0."""
            jcs = tuple(range(GRP * grp, GRP * grp + GRP))
            for jc in jcs:
                e = es[jc]
                for d in range(D):
                    nc.vector.copy_predicated(
                        e[:], mfs[(jc, d)][:].bitcast(dt.uint16), zs[(jc, d)][:])

        def stage_act(grp):
            """Lrelu + Exp batched per function (2 LUT swaps per group)."""
            jcs = tuple(range(GRP * grp, GRP * grp + GRP))
            elr = {}
            for jc in jcs:
                t = ptpool.tile([P, N], dt.float16, name="elr", tag="elr")
                nc.scalar.activation(t[:], es[jc][:], AF.Lrelu, bias=0.0,
                                     scale=1.0, alpha=0.01)
                elr[jc] = t
            for jc in jcs:
                p16 = ptpool.tile([P, N], dt.float16, name="pt", tag="pt")
                nc.scalar.activation(p16[:], elr[jc][:], AF.Exp, bias=0.0,
                                     scale=1.0)
                pt[jc] = p16

        def stage_b(grp):
            """rowsum + S-mults + main matmuls."""
            jcs = tuple(range(GRP * grp, GRP * grp + GRP))
            for jc in jcs:
                for ih in range(2):
                    isl = slice(ih * 512, (ih + 1) * 512)
                    nc.tensor.matmul(rs_ps[:, isl], onescol16[:], pt[jc][:, isl],
                                     start=(jc == 0),
                                     stop=(jc == NJC - 1),
                                     skip_group_check=True)
                for d in range(D):
                    s = spool2.tile([P, N], dt.float16, name=f"s{d}", tag="s")
                    nc.vector.tensor_tensor(
                        s[:], mfs[(jc, d)][:], pt[jc][:], ALU.mult)
                    for ob in range(2):
                        osl = slice(ob * P, (ob + 1) * P)
                        for ih in range(2):
                            isl = slice(ih * 512, (ih + 1) * 512)
                            nc.tensor.matmul(
                                outT_ps[ob][:, isl],
                                whs[(jc, d)][:, osl], s[:, isl],
                                start=(jc == 0 and d == 0),
                                stop=(jc == NJC - 1 and d == D - 1),
                                skip_group_check=True)

        stage_dma(0)
        stage_proj(0)
        stage_z(0)
        stage_merge(0)
        for grp in range(ngrp):
            if grp + 1 < ngrp:
                stage_dma(grp + 1)
                stage_proj(grp + 1)
                stage_z(grp + 1)
            stage_act(grp)
            if grp + 1 < ngrp:
                stage_merge(grp + 1)
            stage_b(grp)

        # ---- epilogue: raw out^T and raw rowsum to HBM; host does the
        # 0.5/(rowsum - corr) scaling and the transpose (free).
        loopctx.close()
        with tc.tile_pool(name="epil", bufs=1) as epil:
            rs_row = epil.tile([1, N], dt.float32, name="rsrow", tag="rsrow")
            nc.vector.tensor_copy(rs_row[:], rs_ps[:])
            nc.sync.dma_start(rsum[:, :], rs_row[:])
            for ob in range(2):
                o = epil.tile([P, N], dt.float32, name=f"osb{ob}", tag=f"osb{ob}")
                nc.scalar.copy(o[:], outT_ps[ob][:])
                nc.sync.dma_start(outT[ob * P:(ob + 1) * P, :], o[:])

    return nc


_CACHED = {}


def _get_nc():
    if "nc" not in _CACHED:
        nc = bass.Bass()
        _build(nc)
        _hoist_waits(nc)
        _dedupe_ldweights(nc)
        _CACHED["nc"] = nc
    return _CACHED["nc"]


def _prep_host(x, A_U, A_D, A_R, A_L, W, a):
    x = np.asarray(x, dtype=np.float32)
    W = np.asarray(W, dtype=np.float32)
    a = np.asarray(a, dtype=np.float32)

    masks = [np.asarray(m) for m in (A_U, A_D, A_R, A_L)]
    # transposed masks (j on rows): mT[d][j, i] = A_d[i, j]
    mT = np.stack([np.ascontiguousarray(m.T) for m in masks])
    m_f16 = (mT != 0).astype(np.float16)

    # attention vector folding (last head only survives the merge)
    wv_cols = [W[d, H - 1] @ a[d, H - 1, :OUT] for d in range(D)] + \
              [W[d, H - 1] @ a[d, H - 1, OUT:] for d in range(D)]
    wvec = np.stack(wv_cols, axis=1).astype(np.float32)   # (F, 2D)
    ws = np.ascontiguousarray(W.sum(axis=1), dtype=np.float32)  # (D, F, OUT)
    return x, m_f16, wvec, ws


def kernel(x, A_U, A_D, A_R, A_L, W, a):
    x, m_f16, wvec, ws = _prep_host(x, A_U, A_D, A_R, A_L, W, a)

    nc = _get_nc()
    core_ids = list(range(B))
    in_maps = []
    for b in range(B):
        in_maps.append({
            "xT": np.ascontiguousarray(x[b].T),
            "mF16": m_f16,
            "wvec": wvec,
            "ws": ws,
        })
    res = run_bass_kernel_spmd(nc, in_maps, core_ids)
    outs = []
    for b in range(B):
        ot = res.results[b]["outT"].astype(np.float64)     # (OUT, N) raw
        rs = res.results[b]["rsum"].astype(np.float64)[0]  # (N,) raw
        outs.append((ot * (0.5 / rs)[None, :]).T)
    return np.stack(outs, axis=0).astype(np.float32)
